# revision 22
# baseline (speedup 1.0000x reference)
"""Trainium2 Bass kernel for quantized 3x3 conv2d (stride 1, pad 1).

Reference computes: conv2d(quant16(x), quant16(w)) where quant16 rounds to
signed 16-bit fixed point with 12 fractional bits (round-half-even, /4096).

Strategy (per core, data-parallel over batch: 4 images/core on 8 cores):
  - Quantize on device with the magic-number trick (+1.5*2^23 in f32 RNE)
    giving rx = round(x*4096) exactly (round-half-even, matches jnp.round).
  - rx needs 16 bits; fp16 holds 11-bit mantissas, so split rx into two
    exact fp16 terms: Xh = fp16(rx) (RNE), Xl = rx - Xh (|Xl| <= 16).
    rw = round(w*4096) fits fp16 exactly (|rw| ~ 1100 < 2048).
  - 3x3 conv = 9 shifted matmuls accumulating in PSUM over a zero-padded
    58x58 image laid out [Cin=128 partitions, 58*58]. Contraction dim =
    partition dim = Cin = 128. Cout=256 -> two 128-row output chunks.
    2 fp16 terms x 9 taps x 2 Cout halves accumulate per output tile.
  - PSUM result = 2^24 * conv(qx, qw); the PSUM->SBUF eviction copy applies
    the 2^-24 scale for free (ScalarE activation Copy with scale).
  - Products are exact in fp32 (11x12-bit mantissas), so accuracy matches
    the f32 reference up to accumulation order.
"""

import numpy as np

B, CIN, COUT, H, W = 32, 128, 256, 56, 56
NCORES = 8
BL = B // NCORES          # images per core
HP = H + 2                # padded height/width (58)
NPIX = H * W              # 3136
NPAD = HP * HP            # 3364
SCALE = 4096.0
MAGIC = 12582912.0        # 1.5 * 2**23: f32 add forces round-to-nearest-even at ulp=1
OSCALE = 1.0 / (SCALE * SCALE)
GROUP_ROWS = 7            # output rows per PSUM tile
NGRP = H // GROUP_ROWS    # 8 groups of 392 px
GRP_PIX = GROUP_ROWS * W  # 392
ROUND_PIX = 4 * GRP_PIX   # 1568 px per PSUM round (4 banks)

_cache = {}


def _build():
    import concourse.bacc as bacc
    import concourse.mybir as mybir
    import concourse.tile as tile

    f32, f16 = mybir.dt.float32, mybir.dt.float16
    Copy = mybir.ActivationFunctionType.Copy
    Alu = mybir.AluOpType

    nc = bacc.Bacc("TRN2", target_bir_lowering=False)
    # x arrives zero-padded to 58x58 from the host so every DMA is contiguous
    x_in = nc.dram_tensor("x", [BL, CIN, NPAD], f32, kind="ExternalInput")
    w_in = nc.dram_tensor("w", [CIN, 9 * COUT], f32, kind="ExternalInput")
    out = nc.dram_tensor("out", [BL, COUT, NPIX], f32, kind="ExternalOutput")

    with tile.TileContext(nc) as tc:
        with (
            tc.tile_pool(name="fixed", bufs=1) as fx,
            tc.tile_pool(name="psum", bufs=1, space="PSUM") as pp,
        ):
            # ---- per-image ping-pong buffers ----
            xsts = [fx.tile([CIN, NPAD], f32, name=f"xst{i}") for i in range(2)]
            ts = [fx.tile([CIN, NPAD], f32, name=f"t{i}") for i in range(2)]
            xhs = [fx.tile([CIN, NPAD], f16, name=f"xh{i}") for i in range(2)]
            xh32s = [fx.tile([CIN, NPAD], f32, name=f"xh32_{i}") for i in range(2)]
            xls = [fx.tile([CIN, NPAD], f16, name=f"xl{i}") for i in range(2)]
            osbs = [fx.tile([128, ROUND_PIX], f32, name=f"osb{i}") for i in range(3)]
            ps = [pp.tile([128, GRP_PIX], f32, name=f"ps{i}") for i in range(8)]
            wst = fx.tile([CIN, 9 * COUT], f32)
            wt = fx.tile([CIN, 9 * COUT], f32)
            w16 = fx.tile([CIN, 9 * COUT], f16)

            # Staging is split into two row-chunks so the quantize chain (and
            # the first PE round) starts before the whole image has landed.
            # Chunk 0 = padded rows [0, 30) (everything PE rounds half=0 read),
            # chunk 1 = padded rows [30, 58).
            CHUNKS = [(0, 30), (30, HP)]

            def stage_chunk(b, c):
                s = b % 2
                xst, t, xh, xh32, xl = xsts[s], ts[s], xhs[s], xh32s[s], xls[s]
                r0, r1 = CHUNKS[c]
                lo, hi = r0 * HP, r1 * HP
                nc.sync.dma_start(out=xst[:, lo:hi], in_=x_in[b, :, lo:hi])
                # t = x*4096 + MAGIC  (exact fma; the add performs RNE rounding)
                nc.scalar.activation(t[:, lo:hi], xst[:, lo:hi], Copy, bias=MAGIC, scale=SCALE)
                # Xh = fp16(rx)  (f32 subtract exact, fp16 convert-on-write RNE)
                nc.vector.tensor_scalar_add(xh[:, lo:hi], t[:, lo:hi], -MAGIC)
                nc.scalar.activation(xh32[:, lo:hi], xh[:, lo:hi], Copy)
                # Xl = rx - Xh  (exact, |Xl| <= 16)
                nc.vector.scalar_tensor_tensor(
                    xl[:, lo:hi], t[:, lo:hi], -MAGIC, xh32[:, lo:hi],
                    Alu.add, Alu.subtract,
                )

            # ---- weights: load + quantize to fp16 integers (rw = round(w*4096)) ----
            # ch-major layout [ci, (ch, tap, co)]; the ch=0 half stages first
            # so the first LDWEIGHTS only waits for half the weight bytes.
            # Image-0 chunk-0 is issued first: its chain is the longest pole
            # to the first matmul.
            HW_COLS = 9 * 128  # 1152 columns per cout-half
            stage_chunk(0, 0)
            for wc in range(2):
                lo, hi = wc * HW_COLS, (wc + 1) * HW_COLS
                nc.sync.dma_start(out=wst[:, lo:hi], in_=w_in[:, lo:hi])
                # rw+MAGIC then -MAGIC, both on DVE (two-op tensor_scalar)
                # to keep the ACT queue free for the image-0 chain
                nc.vector.tensor_scalar(
                    out=wt[:, lo:hi], in0=wst[:, lo:hi],
                    scalar1=SCALE, scalar2=MAGIC,
                    op0=Alu.mult, op1=Alu.add,
                )
                nc.vector.tensor_scalar_add(w16[:, lo:hi], wt[:, lo:hi], -MAGIC)
                if wc == 0:
                    stage_chunk(0, 1)
            stage_chunk(1, 0)
            stage_chunk(1, 1)

            rnd = 0
            for b in range(BL):
                if b >= 2:
                    stage_chunk(b, 0)
                    stage_chunk(b, 1)
                s = b % 2
                xh3 = xhs[s][:].rearrange("p (h w) -> p h w", h=HP)
                xl3 = xls[s][:].rearrange("p (h w) -> p h w", h=HP)

                for ch in range(2):
                    for half in range(2):
                        bank = (rnd % 2) * 4
                        osb = osbs[rnd % 3]
                        # First round: all-Xh taps first so the PE can start
                        # before Xl is staged. Steady state: taps outer so 8
                        # consecutive matmuls share one stationary weight.
                        if rnd == 0:
                            seq = [(tap, term) for term in range(2) for tap in range(9)]
                        else:
                            seq = [(tap, term) for tap in range(9) for term in range(2)]
                        for si, (tap, term) in enumerate(seq):
                            dh, dw = divmod(tap, 3)
                            wsl = w16[:, ch * 1152 + tap * 128 : ch * 1152 + tap * 128 + 128]
                            xt3 = xh3 if term == 0 else xl3
                            for g in range(4):
                                r0 = (half * 4 + g) * GROUP_ROWS
                                mv = xt3[:, r0 + dh : r0 + dh + GROUP_ROWS, dw : dw + W]
                                nc.tensor.matmul(
                                    ps[bank + g][:],
                                    wsl,
                                    mv,
                                    start=(si == 0),
                                    stop=(si == 17),
                                )
                        last_round = rnd == BL * 4 - 1
                        if last_round:
                            # spread the tail: drains split ACT/DVE, per-bank
                            # stores so the final DMA isn't one serial lump
                            for g in range(4):
                                dst = osb[:, g * GRP_PIX : (g + 1) * GRP_PIX]
                                if g % 2 == 0:
                                    nc.scalar.activation(dst, ps[bank + g][:], Copy, scale=OSCALE)
                                else:
                                    nc.vector.tensor_scalar_mul(dst, ps[bank + g][:], OSCALE)
                                nc.sync.dma_start(
                                    out=out[
                                        b,
                                        ch * 128 : (ch + 1) * 128,
                                        half * ROUND_PIX + g * GRP_PIX : half * ROUND_PIX + (g + 1) * GRP_PIX,
                                    ],
                                    in_=dst,
                                )
                        else:
                            for g in range(4):
                                nc.scalar.activation(
                                    osb[:, g * GRP_PIX : (g + 1) * GRP_PIX],
                                    ps[bank + g][:],
                                    Copy,
                                    scale=OSCALE,
                                )
                            nc.sync.dma_start(
                                out=out[
                                    b,
                                    ch * 128 : (ch + 1) * 128,
                                    half * ROUND_PIX : (half + 1) * ROUND_PIX,
                                ],
                                in_=osb[:],
                            )
                        rnd += 1
    nc.compile()
    return nc


def _get_nc():
    if "nc" not in _cache:
        _cache["nc"] = _build()
    return _cache["nc"]


def _maybe_install_trace_bridge():
    """Optional: bridge antenv.axon_hooks so trace=True can capture NTFF."""
    import sys
    import types

    if "antenv.axon_hooks" in sys.modules:
        return
    try:
        from trn_agent_boot.trn_boot import _ntff_profile_via_ctypes

        hook = _ntff_profile_via_ctypes("/opt/axon/libaxon_pjrt.so")
        mod = types.ModuleType("antenv.axon_hooks")
        mod.get_axon_ntff_profile_hook = lambda: hook
        mod.set_axon_ntff_profile_hook = lambda h: None
        import antenv

        sys.modules["antenv.axon_hooks"] = mod
        antenv.axon_hooks = mod
    except Exception:
        pass


def kernel(**inputs):
    import os

    from concourse.bass_utils import run_bass_kernel_spmd

    x = np.ascontiguousarray(np.asarray(inputs["x"], dtype=np.float32))
    weight = np.ascontiguousarray(np.asarray(inputs["weight"], dtype=np.float32))
    assert x.shape == (B, CIN, H, W), x.shape
    assert weight.shape == (COUT, CIN, 3, 3), weight.shape

    # [Cout, Cin, kh, kw] -> [Cin, (ch, kh kw, co128)] so each (ch, tap)
    # slice is a ready [K=ci, M=co] stationary operand, ch-major so the
    # kernel can stage the ch=0 half first.
    w_r = np.ascontiguousarray(
        weight.reshape(2, 128, CIN, 9)
        .transpose(2, 0, 3, 1)
        .reshape(CIN, 9 * COUT)
    )
    xp = np.zeros((B, CIN, HP, HP), dtype=np.float32)
    xp[:, :, 1 : 1 + H, 1 : 1 + W] = x.reshape(B, CIN, H, W)
    xp = xp.reshape(B, CIN, NPAD)
    in_maps = [
        {"x": xp[i * BL : (i + 1) * BL], "w": w_r}
        for i in range(NCORES)
    ]

    trace = bool(int(os.environ.get("KERNEL_TRACE", "0")))
    if trace:
        _maybe_install_trace_bridge()
    nc = _get_nc()
    res = run_bass_kernel_spmd(nc, in_maps, core_ids=list(range(NCORES)), trace=trace)
    _cache["exec_time_ns"] = res.exec_time_ns
    _cache["res"] = res

    outs = [res.results[i]["out"].reshape(BL, COUT, H, W) for i in range(NCORES)]
    return np.concatenate(outs, axis=0)
